# revision 40
# baseline (speedup 1.0000x reference)
"""Trainium2 Bass kernel for top-2 MoE (nn_MoE_2113123910117).

Strategy (expert-parallel with FF-split load balancing):
  - Host: router logits -> softmax -> top-2 -> normalized combine weights.
  - Work is split into 16 shards: (expert e, ff-half h), each covering the
    expert's routed tokens x one half of the FF dimension (SwiGLU is
    elementwise in f, and the down projection is linear in f, so ff-halves
    produce independent partial outputs that the host scatter-ADD combines).
  - Shards are paired onto 8 cores (largest with smallest), giving every
    core the same two-segment program shape (G1-group + G2-group segments);
    token-group imbalance drops from 17 full-FF groups to 16.5 equivalents.
  - Device (per core, per segment): y_partial = diag(s) @
    ((silu(x WgT_half) * (x WuT_half)) WdT_half), fp16 matmuls with fp32
    PSUM accumulation.
  - Host: scatter-add per-shard partial outputs into the [B,T,D] output.

Schedule notes:
  - Gate/up weights are host-packed per f-tile (row block f of wgu =
    [128, 2*D] holding 8 gate d-blocks then 8 up d-blocks) so the PE can
    start the first gate chain after ~0.8MB of DMA instead of the full
    gate-weight stream; x is packed [128, ND, CAP] so each x tile is one
    DMA descriptor (descriptors cost ~625ns of HWDGE each).
  - DMA order is the startup critical path: wguA[0] (gate half first),
    x tile 0 (two halves), rest of wguA, sc, x1, wdA, x2, then segment B's
    weights; later x tiles prefetch 3 ahead through a triple-buffered pool
    (crossing the segment boundary without a stall).
  - A burst of zero matmuls issues before the weights arrive: the PE is
    idle until ~5us anyway and the activity un-throttles the HAM clock
    gate (1.2 -> 2.4 GHz) before the real work starts.
  - Token tiles are balanced multiples of 128 with max 512 (PSUM bank cap)
    and min 384; the per-token combine scale is applied by the Vector
    engine (tensor_scalar_mul; GPSIMD cannot read PSUM), y returns fp16.

Self-contained: hardcodes all shapes from the problem spec.
"""

import os
import numpy as np

D = 1024
FF = 2048
E = 8
TOPK = 2
NCORES = 8
ND = D // 128     # 8 contraction chunks (gate/up)
NFH = FF // 2 // 128  # 8 ff chunks per half-shard
MIN_G = (17, 16)  # segment group counts for the spec'd input

# matmul operand dtype on device ("float16", "bfloat16")
MM_DTYPE = os.environ.get("MOE_MM_DTYPE", "float16")

# test-only knobs / results (harness never touches these)
LAST_RESULTS = None
_NC_CACHE = {}


def split_multi_waits(nc, mybir_mod):
    """This walrus build rejects any instruction carrying more than one
    sync wait ("Too many sync wait commands"). Hoist extra waits onto
    single-wait NOPs inserted just before the instruction on the same
    engine — semantically identical since engines execute in order."""
    n_split = 0
    for f in nc.m.functions:
        for blk in f.blocks:
            insts = blk.instructions
            newl = []
            changed = False
            for inst in insts:
                si = inst.sync_info
                if si is not None and len(si.on_wait) > 1:
                    waits = list(si.on_wait)
                    del si.on_wait[1:]
                    for j, w in enumerate(waits[1:]):
                        nop = mybir_mod.InstNoOp(
                            name=f"{inst.name}_w{j}",
                            engine=inst.engine,
                            ins=[],
                            outs=[],
                        )
                        nop.sync_info = mybir_mod.SyncInfo(on_wait=[w], on_update=[])
                        newl.append(nop)
                        n_split += 1
                    changed = True
                newl.append(inst)
            if changed:
                insts[:] = newl
    return n_split


def _token_tiles(ngroups):
    """Balanced token tiles (in groups of 128): each tile <= 4 groups,
    sized as evenly as possible, smallest first."""
    ntiles = -(-ngroups // 4)
    base, extra = divmod(ngroups, ntiles)
    sizes = [base + (1 if i >= ntiles - extra else 0) for i in range(ntiles)]
    tiles = []
    off = 0
    for g in sizes:
        tiles.append((off, g * 128))
        off += g * 128
    return tiles


def build_nc(segs, repeat=1):
    """Per-core Bass program: two half-FF FFN segments with group counts
    ``segs = (G1, G2)`` over zero-padded token buffers. Same NEFF on all
    8 cores (SPMD).

    repeat>1 wraps the body in a hardware loop (benchmark-only); the PE
    pre-warm stays outside the loop so the proxy measures the real body."""
    import contextlib

    import concourse.bass as bass
    import concourse.mybir as mybir
    import concourse.tile as tile

    dt = mybir.dt
    f32 = dt.float32
    mmdt = getattr(dt, MM_DTYPE)
    AF = mybir.ActivationFunctionType
    G1, G2 = segs
    NG = G1 + G2                     # total token groups across segments
    CAP = NG * 128

    nc = bass.Bass()
    # x^T in [partition, d-chunk, token] layout: one descriptor per x-tile
    xt = nc.dram_tensor("xt", [128, ND, CAP], mmdt, kind="ExternalInput")
    # rows [s*1024 + f*128 ...]: f-tile f of segment s (gate cols 0:D, up D:2D)
    wgu = nc.dram_tensor("wgu", [2 * NFH * 128, 2 * D], mmdt, kind="ExternalInput")
    # rows [s*1024 ...]: segment s's half of Wd^T ([FF/2, D])
    wd = nc.dram_tensor("wd", [2 * NFH * 128, D], mmdt, kind="ExternalInput")
    sc = nc.dram_tensor("sc", [128, NG], f32, kind="ExternalInput")
    y = nc.dram_tensor("y", [CAP, D], mmdt, kind="ExternalOutput")

    # flat tile list across segments: (seg, group_offset_global, ntok)
    # segment A leads with a small 2-group tile: less x-DMA gates the
    # very first matmul chain
    all_tiles = []
    for s, G in enumerate(segs):
        goff = 0 if s == 0 else G1
        if s == 0 and G > 6:
            sizes = [(0, 256)] + [(256 + o, t) for o, t in _token_tiles(G - 2)]
        else:
            sizes = _token_tiles(G)
        for off, tt in sizes:
            all_tiles.append((s, goff * 128 + off, tt))

    with tile.TileContext(nc) as tc:
        with (
            tc.tile_pool(name="wpool", bufs=1) as wpool,
            tc.tile_pool(name="x0pool", bufs=1) as x0pool,
            tc.tile_pool(name="xpool", bufs=3) as xpool,
            tc.tile_pool(name="hpool", bufs=2) as hpool,
            tc.tile_pool(name="gpool", bufs=3) as gpool,
            tc.tile_pool(name="ypool", bufs=3) as ypool,
            tc.tile_pool(name="pg", bufs=2, space="PSUM") as pgpool,
            tc.tile_pool(name="pu", bufs=2, space="PSUM") as pupool,
            tc.tile_pool(name="po", bufs=4, space="PSUM") as popool,
        ):
            # PE pre-warm (outside the benchmark repeat loop — only the cold
            # first pass benefits): the HAM clock gate needs ~3.4us of PE
            # activity to unthrottle 1.2->2.4GHz. The PE idles until the
            # first weights arrive (~5us) anyway, so a zero matmul burst
            # warms it for free (zeros keep CoreSim's finite-checks happy).
            zw = wpool.tile([128, 512], mmdt, tag="zwarm")
            nc.vector.memset(zw[:], 0)
            pwarm = popool.tile([128, 512], f32, tag="po", name="po_warm")
            for _ in range(7):
                nc.tensor.matmul(pwarm[:], zw[:, 0:128], zw[:],
                                 start=True, stop=True)

            loop_cm = (
                tc.For_i(0, repeat, 1, hint_engines=(mybir.EngineType.PE,))
                if repeat > 1
                else contextlib.nullcontext()
            )
            with loop_cm:
                # --- DMA order is the startup critical path ---
                wgu_sb = [[None] * NFH for _ in range(2)]
                wd_sb = [[None] * NFH for _ in range(2)]

                def load_wgu(s, f, split=False):
                    t = wpool.tile([128, 2 * D], mmdt, tag=f"wgu{s}_{f}",
                                   name=f"wgu_sb{s}_{f}")
                    r = s * NFH * 128 + f * 128
                    if split:
                        # gate half first: unblocks the first gate chain sooner
                        nc.sync.dma_start(t[:, 0:D], wgu[r : r + 128, 0:D])
                        nc.sync.dma_start(t[:, D : 2 * D],
                                          wgu[r : r + 128, D : 2 * D])
                    else:
                        nc.sync.dma_start(t[:], wgu[r : r + 128, :])
                    wgu_sb[s][f] = t

                def load_wd(s, f):
                    t = wpool.tile([128, D], mmdt, tag=f"wd{s}_{f}",
                                   name=f"wd_sb{s}_{f}")
                    r = s * NFH * 128 + f * 128
                    nc.sync.dma_start(t[:], wd[r : r + 128, :])
                    wd_sb[s][f] = t

                def load_x(tile_idx):
                    eng = nc.sync
                    _, toff, tt = all_tiles[tile_idx]
                    if tile_idx == 0:
                        # two half-loads: first gate chain starts mid-stream
                        xs = []
                        for half in range(2):
                            t = x0pool.tile([128, ND // 2, tt], mmdt,
                                            tag=f"xt{half}",
                                            name=f"x_{tile_idx}_{half}")
                            eng.dma_start(
                                t[:],
                                xt[:, half * (ND // 2) : (half + 1) * (ND // 2),
                                   toff : toff + tt])
                            xs.extend(t[:, d, :] for d in range(ND // 2))
                        return xs
                    t = xpool.tile([128, ND, tt], mmdt, tag="xb",
                                   name=f"x_{tile_idx}")
                    eng.dma_start(t[:], xt[:, :, toff : toff + tt])
                    return [t[:, d, :] for d in range(ND)]

                # interleave the first x half ahead of the gate weights:
                # first MM waits max(wgu00-gate, x0h0), and each later
                # consumer's data still lands before its first use
                _, toff0, tt0 = all_tiles[0]
                x0_tiles = []
                for half in range(2):
                    t = x0pool.tile([128, ND // 2, tt0], mmdt,
                                    tag=f"xt{half}", name=f"x_0_{half}")
                    x0_tiles.append(t)
                t00 = wpool.tile([128, 2 * D], mmdt, tag="wgu0_0",
                                 name="wgu_sb0_0")
                wgu_sb[0][0] = t00
                nc.sync.dma_start(
                    x0_tiles[0][:], xt[:, 0 : ND // 2, toff0 : toff0 + tt0])
                nc.sync.dma_start(t00[:, 0:D], wgu[0:128, 0:D])
                nc.sync.dma_start(
                    x0_tiles[1][:], xt[:, ND // 2 : ND, toff0 : toff0 + tt0])
                nc.sync.dma_start(t00[:, D : 2 * D], wgu[0:128, D : 2 * D])
                x_sb = {0: [t[:, d, :] for t in x0_tiles
                            for d in range(ND // 2)]}
                for f in range(1, NFH):
                    load_wgu(0, f)
                s_sb = wpool.tile([128, NG], f32, tag="s")
                nc.sync.dma_start(s_sb[:], sc[:])
                x_sb[1] = load_x(1)
                for f in range(NFH):
                    load_wd(0, f)
                x_sb[2] = load_x(2)
                for f in range(NFH):
                    load_wgu(1, f)
                for f in range(NFH):
                    load_wd(1, f)

                for ti, (s, toff, tt) in enumerate(all_tiles):
                    xt_t = x_sb.pop(ti)
                    if ti + 3 < len(all_tiles):
                        x_sb[ti + 3] = load_x(ti + 3)  # 3-buffered prefetch
                    # gate/up + SwiGLU -> h^T [f, tokens]
                    ht_t = []
                    for f in range(NFH):
                        pg = pgpool.tile([128, tt], f32, tag="pg")
                        pu = pupool.tile([128, tt], f32, tag="pu")
                        for d in range(ND):
                            nc.tensor.matmul(
                                pg[:],
                                wgu_sb[s][f][:, d * 128 : (d + 1) * 128],
                                xt_t[d],
                                start=(d == 0),
                                stop=(d == ND - 1),
                            )
                        for d in range(ND):
                            nc.tensor.matmul(
                                pu[:],
                                wgu_sb[s][f][:, D + d * 128 : D + (d + 1) * 128],
                                xt_t[d],
                                start=(d == 0),
                                stop=(d == ND - 1),
                            )
                        sg = gpool.tile([128, tt], mmdt, tag="sg")
                        nc.scalar.activation(sg[:], pg[:], AF.Silu)
                        ht = hpool.tile([128, tt], mmdt, tag=f"ht{f}")
                        nc.vector.tensor_mul(ht[:], sg[:], pu[:])
                        ht_t.append(ht)
                    # down projection (partial: half the ff contraction),
                    # scaled by combine weight per token
                    for k in range(tt // 128):
                        g = toff // 128 + k
                        po_h = []
                        for dh in range(2):
                            po = popool.tile([128, 512], f32, tag="po",
                                             name=f"po_{toff}_{k}_{dh}")
                            po_h.append(po)
                        for f in range(NFH):
                            lhs = ht_t[f][:, k * 128 : (k + 1) * 128]
                            for dh in range(2):
                                nc.tensor.matmul(
                                    po_h[dh][:],
                                    lhs,
                                    wd_sb[s][f][:, dh * 512 : (dh + 1) * 512],
                                    start=(f == 0),
                                    stop=(f == NFH - 1),
                                )
                        last = (ti == len(all_tiles) - 1) and (k == tt // 128 - 1)
                        yt = ypool.tile([128, D], mmdt, tag="yt")
                        if last:
                            # tail: scale-muls on two engines, split stores
                            # so each half fires immediately
                            nc.vector.tensor_scalar_mul(
                                yt[:, 0:512], po_h[0][:], s_sb[:, g : g + 1])
                            nc.scalar.activation(
                                yt[:, 512:1024], po_h[1][:], AF.Copy,
                                scale=s_sb[:, g : g + 1])
                            for dh in range(2):
                                nc.sync.dma_start(
                                    y[toff + k * 128 : toff + (k + 1) * 128,
                                      dh * 512 : (dh + 1) * 512],
                                    yt[:, dh * 512 : (dh + 1) * 512])
                        else:
                            for dh in range(2):
                                nc.vector.tensor_scalar_mul(
                                    yt[:, dh * 512 : (dh + 1) * 512],
                                    po_h[dh][:], s_sb[:, g : g + 1]
                                )
                            nc.sync.dma_start(
                                y[toff + k * 128 : toff + (k + 1) * 128, :],
                                yt[:]
                            )
    split_multi_waits(nc, mybir)
    return nc


def _get_nc(segs):
    key = (segs, MM_DTYPE)
    if key not in _NC_CACHE:
        _NC_CACHE[key] = build_nc(segs)
    return _NC_CACHE[key]


def _route(xf, Wr):
    """fp32 softmax + top-2 + normalized combine weights, matching the
    jax reference (ties broken toward lower expert index)."""
    logits = xf @ Wr.astype(np.float32).T
    m = logits.max(-1, keepdims=True)
    ex = np.exp(logits - m)
    p = ex / ex.sum(-1, keepdims=True)
    top2 = np.argsort(-p, axis=-1, kind="stable")[:, :TOPK]
    n = xf.shape[0]
    p1 = p[np.arange(n), top2[:, 0]]
    p2 = p[np.arange(n), top2[:, 1]]
    denom = (p1 + p2) + np.float32(1e-8)
    return top2, p1 / denom, p2 / denom


def _pack_wgu_half(Wg_half, Wu_half, mmnp):
    """Pack one ff-half of gate/up weights per f-tile:
    out[f*128+p, d*128+c] = W^T[d*128+p, f*128+c], gate in cols [0,D),
    up in cols [D,2D). W*_half: [FF/2, D]."""
    nf = Wg_half.shape[0] // 128
    out = np.empty((nf * 128, 2 * D), dtype=mmnp)
    for half, W in ((0, Wg_half), (1, Wu_half)):
        WT = np.ascontiguousarray(W.T).astype(mmnp)          # [D, FF/2]
        A = WT.reshape(ND, 128, nf, 128)                     # [d, p, f, c]
        out[:, half * D : (half + 1) * D] = (
            A.transpose(2, 1, 0, 3).reshape(nf * 128, D)
        )
    return out


def make_in_maps(x, Wr, Wg, Wu, Wd):
    """Route on host, pair (expert, ff-half) shards onto cores, build
    per-core device input maps. Returns (in_maps, assignments, segs, n_tok)
    where assignments[c] = [(e, h, n_e, idx_e), (e, h, n_e, idx_e)]."""
    xf = x.reshape(-1, D).astype(np.float32, copy=False)
    top2, s1, s2 = _route(xf, Wr)

    mmnp = np.dtype(np.float16 if MM_DTYPE == "float16" else np.float32)
    if MM_DTYPE == "bfloat16":
        import ml_dtypes

        mmnp = np.dtype(ml_dtypes.bfloat16)

    xf_mm = xf.astype(mmnp)

    idxs, scs, groups = [], [], []
    for e in range(E):
        idx = np.nonzero((top2[:, 0] == e) | (top2[:, 1] == e))[0]
        idxs.append(idx)
        scs.append(np.where(top2[idx, 0] == e, s1[idx], s2[idx]).astype(np.float32))
        groups.append(max(1, -(-len(idx) // 128)))

    # shards (e, h) sorted by size desc; pair i-th largest with i-th smallest
    shards = sorted(
        [(e, h) for e in range(E) for h in range(2)],
        key=lambda eh: (-groups[eh[0]], eh[0], eh[1]),
    )
    assignments = []
    for c in range(NCORES):
        assignments.append([shards[c], shards[2 * NCORES - 1 - c]])
    G1 = max(groups[a[0][0]] for a in assignments)
    G2 = max(groups[a[1][0]] for a in assignments)
    G1, G2 = max(G1, MIN_G[0]), max(G2, MIN_G[1])
    segs = (G1, G2)
    CAP = (G1 + G2) * 128

    in_maps = []
    asg_meta = []
    for c in range(NCORES):
        xt = np.zeros((128, ND, CAP), dtype=mmnp)
        wgu = np.empty((2 * NFH * 128, 2 * D), dtype=mmnp)
        wd = np.empty((2 * NFH * 128, D), dtype=mmnp)
        scv = np.zeros(CAP, dtype=np.float32)
        meta = []
        for s, (e, h) in enumerate(assignments[c]):
            idx = idxs[e]
            n_e = len(idx)
            coff = 0 if s == 0 else G1 * 128
            xt[:, :, coff : coff + n_e] = (
                xf_mm[idx].T.reshape(ND, 128, n_e).transpose(1, 0, 2)
            )
            scv[coff : coff + n_e] = scs[e]
            rows = slice(s * NFH * 128, (s + 1) * NFH * 128)
            frows = slice(h * (FF // 2), (h + 1) * (FF // 2))
            wgu[rows] = _pack_wgu_half(Wg[e][frows], Wu[e][frows], mmnp)
            wd[rows] = np.ascontiguousarray(Wd[e].T[frows]).astype(mmnp)
            meta.append((e, h, n_e))
        sc2d = np.ascontiguousarray(scv.reshape(-1, 128).T)
        in_maps.append({"xt": xt, "wgu": wgu, "wd": wd, "sc": sc2d})
        asg_meta.append(meta)
    return in_maps, asg_meta, segs, xf.shape[0], idxs


def kernel(**inputs):
    global LAST_RESULTS
    from concourse.bass_utils import run_bass_kernel_spmd

    x = np.asarray(inputs["x"])
    B, T, _ = x.shape
    in_maps, asg, segs, n_tok, idxs = make_in_maps(
        x, np.asarray(inputs["Wr"]), np.asarray(inputs["Wg"]),
        np.asarray(inputs["Wu"]), np.asarray(inputs["Wd"]),
    )

    nc = _get_nc(segs)
    res = run_bass_kernel_spmd(nc, in_maps, list(range(NCORES)))
    LAST_RESULTS = res

    out = np.zeros((n_tok, D), dtype=np.float32)
    G1 = segs[0]
    for c in range(NCORES):
        yc = res.results[c]["y"]
        for s, (e, h, n_e) in enumerate(asg[c]):
            coff = 0 if s == 0 else G1 * 128
            out[idxs[e]] += yc[coff : coff + n_e]
    return out.reshape(B, T, D).astype(x.dtype, copy=False)
